# revision 49
# baseline (speedup 1.0000x reference)
"""CCPL contrastive loss kernel for Trainium2 (8 NeuronCores, SPMD data-parallel over batch).

Contract: kernel(**inputs) takes the FULL unsharded inputs and returns the FULL
scalar loss (float32, shape ()).

Strategy
--------
Only the top-left 32x32 corner of each feature map is ever read (sid in [0,30),
neighborhood offsets in {0,1,2}).  The host performs *indexing only* (gather of
neighbor/center columns from the corner; no arithmetic) and uploads, per core:

  xq, xk : [8*128, 576] packed K-chunks of [neigh(512) | center(64)] columns
  wts    : [128, WTOT]  packed transposed MLP weights (w1T / w2T chunks)
  aux    : [128, 268]   b1/b2 columns, identity block, ones block

Core b processes batch b end-to-end on device (fp16 matmul operands,
fp32 PSUM/softmax math):
  x = neigh - center                  (VectorE, stride-0 broadcast AP)
  h = relu(w1 @ x + b1)               (TensorE + ScalarE relu w/ bias AP)
  y = w2 @ h + b2                     (TensorE + VectorE tensor_scalar)
  f = y / ||y||_2                     (ones-matmul partition reduction;
                                       1/sqrt via ScalarE exp(-0.5*ln) --
                                       single exp/ln/relu ACT table set)
  G = f_q^T f_k                       (TensorE; |G|<=1 so exp needs no max)
  sum_s CE[s] = sum ln(sum_t exp(G/tau)) - (1/tau) sum <f_q[:,s],f_k[:,s]>
Layers processed big-first; per-layer tails rotate through fine-grained
1-bank PSUM tiles so the 4 layers' chains pipeline. Per-core partial CE
sum returned as [1,1]; host sums 8 partials / (8*512).
"""

import numpy as np
from contextlib import ExitStack

import concourse.bass as bass
import concourse.bacc as bacc
import concourse.tile as tile
from concourse import mybir
from concourse.bass_utils import run_bass_kernel_spmd

F32 = mybir.dt.float32
F16 = mybir.dt.float16

# Force Exp/Ln/Relu to resolve to the one table set that contains all three
# (natural_log_exp_and_others), so the kernel pays a single ACT_TABLE_LOAD
# instead of thrashing between exp_and_others and natural_log (~1.3us each).
# Set ids stay aligned with act_info.json because only set CONTENTS are
# filtered, never the ordering.
_COMBINED_SET = "natural_log_exp_and_others"
_orig_get_tables = bacc.get_activation_tables


def _patched_get_tables(arch):
    t = _orig_get_tables(arch)
    strip = {
        mybir.ActivationFunctionType.Exp,
        mybir.ActivationFunctionType.Ln,
        mybir.ActivationFunctionType.Relu,
    }
    return {
        name: (fns if name == _COMBINED_SET else (set(fns) - strip))
        for name, fns in t.items()
    }


bacc.get_activation_tables = _patched_get_tables

TAU = 0.07
NCORES = 8
S = 512          # 8 * num_s samples per batch-layer
NS = 64          # num_s
CS = [64, 128, 256, 512]
COUT = [16, 32, 64, 128]
KC = [1, 1, 2, 4]                 # 128-row K chunks per layer
NCH = sum(KC)                     # 8 chunks total in the x blob
_DH = np.array([0, 0, 0, 1, 1, 2, 2, 2], dtype=np.int64)
_DW = np.array([0, 1, 2, 0, 2, 0, 1, 2], dtype=np.int64)

# chunk bookkeeping -----------------------------------------------------------
CHUNK = {}
_c = 0
for _l in range(4):
    for _kk in range(KC[_l]):
        CHUNK[(_l, _kk)] = _c
        _c += 1

# weight blob column offsets, big layers first so the layer-3 block can be
# DMA'd ahead of the rest (it is needed first)
W1C, W2C = {}, {}
_c = 0
for _l in (3, 2, 1, 0):
    for _kk in range(KC[_l]):
        W1C[(_l, _kk)] = _c
        _c += CS[_l]
    for _kk in range(KC[_l]):
        W2C[(_l, _kk)] = _c
        _c += COUT[_l]
    if _l == 3:
        WSPLIT = _c          # end of the layer-3 weight block
WTOT = _c
# contiguous [start, end) column range of each layer's weight block
WBLK = {}
for _l in range(4):
    _s = W1C[(_l, 0)]
    _e = W2C[(_l, KC[_l] - 1)] + COUT[_l]
    WBLK[_l] = (_s, _e)

# aux blob layout (f32): cols 0..7 b1 chunks, 8..11 b2, 12..139 I128, 140..267 ones
B1C = {}
_c = 0
for _l in range(4):
    for _m in range(KC[_l]):
        B1C[(_l, _m)] = _c
        _c += 1
B2C = {l: 8 + l for l in range(4)}
ICOL = 12
OCOL = 140
WVCOL = 268          # row 0: [1.0]*16 | [-1/tau]*4  (final combine weights)
AUXW = 288


def _build_nc(dt_x=F16, mm1_bufs=2, mm2_bufs=2, dma_spread=False,
              layer_order=(3, 2, 1, 0), phase_split=False, fine_psum=False):
    nc = bacc.Bacc()
    xq = nc.dram_tensor("xq", [NCH * 128, 576], dt_x, kind="ExternalInput")
    xk = nc.dram_tensor("xk", [NCH * 128, 576], dt_x, kind="ExternalInput")
    wts = nc.dram_tensor("wts", [128, WTOT], dt_x, kind="ExternalInput")
    aux = nc.dram_tensor("aux", [128, AUXW], F32, kind="ExternalInput")
    auxh = nc.dram_tensor("auxh", [128, 128], F16, kind="ExternalInput")
    out = nc.dram_tensor("out", [1, 1], F32, kind="ExternalOutput")

    with ExitStack() as ctx:
        tc = ctx.enter_context(tile.TileContext(nc))
        const = ctx.enter_context(tc.tile_pool(name="const", bufs=1))
        work = ctx.enter_context(tc.tile_pool(name="work", bufs=2))
        hpool = ctx.enter_context(tc.tile_pool(name="hpool", bufs=3))
        ypool = ctx.enter_context(tc.tile_pool(name="ypool", bufs=6))
        fpool = ctx.enter_context(tc.tile_pool(name="fpool", bufs=6))
        # PSUM budget is 8 banks total:
        # mm1 (1 bank) * mm1_bufs + mm2 (1 bank) * mm2_bufs
        # + gpool bufs=1 * (small [1,2,512] 2 banks + g [128,2,512] 2 banks)
        ppool = ctx.enter_context(
            tc.tile_pool(name="psum", bufs=mm1_bufs, space="PSUM"))
        p2pool = ctx.enter_context(
            tc.tile_pool(name="psum2", bufs=mm2_bufs, space="PSUM"))
        gpool = ctx.enter_context(tc.tile_pool(
            name="gpsum", bufs=(2 if fine_psum else 1), space="PSUM"))

        xq_s = const.tile([128, NCH, 576], dt_x)
        xk_s = const.tile([128, NCH, 576], dt_x)
        wall = const.tile([128, WTOT], dt_x)
        aall = const.tile([128, AUXW], F32)
        hall = const.tile([128, 128], F16)
        rq = xq.rearrange("(n p) m -> p n m", p=128)
        rk = xk.rearrange("(n p) m -> p n m", p=128)
        if dma_spread:
            nc.sync.dma_start(out=xq_s[:, 4:8, :], in_=rq[:, 4:8, :])
            nc.scalar.dma_start(out=xk_s[:, 4:8, :], in_=rk[:, 4:8, :])
            nc.gpsimd.dma_start(out=wall, in_=wts[:, :])
            nc.sync.dma_start(out=xq_s[:, 0:4, :], in_=rq[:, 0:4, :])
            nc.scalar.dma_start(out=xk_s[:, 0:4, :], in_=rk[:, 0:4, :])
            nc.gpsimd.dma_start(out=aall, in_=aux[:, :])
            nc.gpsimd.dma_start(out=hall, in_=auxh[:, :])
        else:
            # land the first-processed layer's x chunk + weight block first,
            # then the rest in processing order
            fl = layer_order[0]
            c0, c1 = CHUNK[(fl, 0)], CHUNK[(fl, 0)] + KC[fl]
            w0, w1_ = WBLK[fl]
            nc.sync.dma_start(out=xq_s[:, c0:c0 + 1, :], in_=rq[:, c0:c0 + 1, :])
            nc.sync.dma_start(out=wall[:, w0:w1_], in_=wts[:, w0:w1_])
            nc.sync.dma_start(out=xk_s[:, c0:c0 + 1, :], in_=rk[:, c0:c0 + 1, :])
            if c1 > c0 + 1:
                nc.sync.dma_start(out=xq_s[:, c0 + 1:c1, :], in_=rq[:, c0 + 1:c1, :])
                nc.sync.dma_start(out=xk_s[:, c0 + 1:c1, :], in_=rk[:, c0 + 1:c1, :])
            nc.sync.dma_start(out=aall, in_=aux[:, :])
            nc.sync.dma_start(out=hall, in_=auxh[:, :])
            for l in layer_order[1:]:
                a0, a1 = CHUNK[(l, 0)], CHUNK[(l, 0)] + KC[l]
                b0, b1_ = WBLK[l]
                nc.sync.dma_start(out=wall[:, b0:b1_], in_=wts[:, b0:b1_])
                nc.sync.dma_start(out=xq_s[:, a0:a1, :], in_=rq[:, a0:a1, :])
                nc.sync.dma_start(out=xk_s[:, a0:a1, :], in_=rk[:, a0:a1, :])

        ones_col = aall[:, OCOL:OCOL + 1]
        # Z (row sums of exp(G/tau)) per G row-tile, one column per tile
        ZD = const.tile([128, 16], F32)
        # catb: cols 0..15 = per-tile sums of ln(Z); cols 16..19 = per-layer
        # sums of l_pos = sum(f_q * f_k)
        catb = const.tile([1, 20], F32)

        # x = neigh - center, four chunks per fused DVE op (center broadcast
        # over the 8 neighbors via a stride-0 trailing AP dim)
        xsub = {}
        sub_slices = []
        for li, l in enumerate(layer_order):
            a0, a1 = CHUNK[(l, 0)], CHUNK[(l, 0)] + KC[l]
            if li == 0:
                # first layer chunk-at-a-time so its first MLP matmul can
                # start as soon as the first chunk has landed
                sub_slices += [slice(c, c + 1) for c in range(a0, a1)]
            else:
                sub_slices.append(slice(a0, a1))
        for bi, xall in enumerate((xq_s, xk_s)):
            xs = const.tile([128, NCH, S], dt_x, tag=f"xsub{bi}")
            for csl in sub_slices:
                in0 = xall[:, csl, 0:512].rearrange("p n (s j) -> p n s j", j=8)
                cb = xall[:, csl, 512:576]
                in1 = bass.AP(cb.tensor, cb.offset, [*cb.ap, [0, 8]])
                nc.vector.tensor_sub(
                    out=xs[:, csl, :].rearrange("p n (s j) -> p n s j", j=8),
                    in0=in0,
                    in1=in1,
                )
            xsub[bi] = xs

        def emit_mlp(l, bi):
            C, Co, K = CS[l], COUT[l], KC[l]
            xs = xsub[bi]
            h = hpool.tile([128, K, S], dt_x, tag="h")
            for m in range(K):
                rows = min(128, C - m * 128)
                mm1 = ppool.tile([128, S], F32, tag="mm1")
                for kk in range(K):
                    c0 = W1C[(l, kk)] + m * 128
                    nc.tensor.matmul(
                        mm1[0:rows, :],
                        lhsT=wall[:, c0:c0 + rows],
                        rhs=xs[:, CHUNK[(l, kk)], :],
                        start=(kk == 0),
                        stop=(kk == K - 1),
                    )
                bc1 = B1C[(l, m)]
                nc.scalar.activation(
                    out=h[0:rows, m, :],
                    in_=mm1[0:rows, :],
                    func=mybir.ActivationFunctionType.Relu,
                    bias=aall[0:rows, bc1:bc1 + 1],
                    scale=1.0,
                )
            mm2 = p2pool.tile([128, S], F32, tag="mm2")
            for kk in range(K):
                rows = min(128, C - kk * 128)
                c0 = W2C[(l, kk)]
                nc.tensor.matmul(
                    mm2[0:Co, :],
                    lhsT=wall[0:rows, c0:c0 + Co],
                    rhs=h[0:rows, kk, :],
                    start=(kk == 0),
                    stop=(kk == K - 1),
                )
            y = ypool.tile([128, S], F32, tag="y")
            nc.vector.tensor_scalar_add(
                out=y[0:Co, :], in0=mm2[0:Co, :],
                scalar1=aall[0:Co, B2C[l]:B2C[l] + 1],
            )
            return y

        def emit_tail_fine(l, ytiles):
            C, Co, K = CS[l], COUT[l], KC[l]
            rns = []
            for bi in range(2):
                y2 = work.tile([128, S], F16, tag="y2")
                nc.gpsimd.tensor_mul(out=y2[0:Co, :], in0=ytiles[bi][0:Co, :],
                                     in1=ytiles[bi][0:Co, :])
                ssq = gpool.tile([1, S], F32, tag="small")
                nc.tensor.matmul(
                    ssq[:, :], lhsT=hall[0:Co, 0:1], rhs=y2[0:Co, :],
                    start=True, stop=True,
                )
                t1 = work.tile([1, S], F32, tag="t1")
                nc.scalar.activation(out=t1[:, :], in_=ssq[:, :],
                                     func=mybir.ActivationFunctionType.Ln)
                rn = work.tile([1, S], F16, tag="rn")
                nc.scalar.activation(out=rn[:, :], in_=t1[:, :],
                                     func=mybir.ActivationFunctionType.Exp,
                                     scale=-0.5)
                rns.append(rn)
            ftiles = []
            for bi in range(2):
                bc = gpool.tile([128, S], F32, tag="gbc")
                nc.tensor.matmul(
                    bc[0:Co, :], lhsT=hall[0:1, 0:Co], rhs=rns[bi][:, :],
                    start=True, stop=True,
                )
                f = fpool.tile([128, S], F16, tag="f")
                nc.vector.tensor_mul(out=f[0:Co, :], in0=ytiles[bi][0:Co, :],
                                     in1=bc[0:Co, :])
                ftiles.append(f)
            fq_t, fk_t = ftiles
            pprod = work.tile([128, S], F16, tag="pprod")
            nc.gpsimd.tensor_mul(out=pprod[0:Co, :], in0=fq_t[0:Co, :],
                                 in1=fk_t[0:Co, :])
            psum_pos = gpool.tile([1, S], F32, tag="small")
            nc.tensor.matmul(psum_pos[:, :], lhsT=hall[0:Co, 0:1],
                             rhs=pprod[0:Co, :], start=True, stop=True)
            nc.vector.reduce_sum(out=catb[:, 16 + l:17 + l],
                                 in_=psum_pos[:, :],
                                 axis=mybir.AxisListType.X)
            for m in range(4):
                g = gpool.tile([128, S], F32, tag="gbc")
                nc.tensor.matmul(
                    g[:, :],
                    lhsT=fq_t[0:Co, m * 128:(m + 1) * 128],
                    rhs=fk_t[0:Co, :],
                    start=True, stop=True,
                )
                E = work.tile([128, S], F32, tag="E")
                nc.scalar.activation(
                    out=E[:, :], in_=g[:, :],
                    func=mybir.ActivationFunctionType.Exp,
                    scale=1.0 / TAU,
                )
                i = l * 4 + m
                nc.vector.reduce_sum(out=ZD[:, i:i + 1], in_=E[:, :],
                                     axis=mybir.AxisListType.X)


        def emit_tail(l, ytiles):
            if fine_psum:
                return emit_tail_fine(l, ytiles)
            C, Co, K = CS[l], COUT[l], KC[l]
            # squared col norms of both branches packed in the free dim of
            # one [1, 2, 512] PSUM tile (2 banks, both MMs partition-base 0)
            ssq = gpool.tile([1, 2, S], F32, tag="small")
            for bi in range(2):
                y2 = work.tile([128, S], F16, tag="y2")
                nc.gpsimd.tensor_mul(out=y2[0:Co, :], in0=ytiles[bi][0:Co, :],
                                     in1=ytiles[bi][0:Co, :])
                nc.tensor.matmul(
                    ssq[:, bi, :], lhsT=hall[0:Co, 0:1], rhs=y2[0:Co, :],
                    start=True, stop=True,
                )
            # rn = 1/sqrt(ssq) = exp(-0.5*ln(ssq)), both branches per ACT op
            t1 = work.tile([1, 2, S], F32, tag="t1")
            nc.scalar.activation(out=t1[:, :, :], in_=ssq[:, :, :],
                                 func=mybir.ActivationFunctionType.Ln)
            rn = work.tile([1, 2, S], F16, tag="rn")
            nc.scalar.activation(out=rn[:, :, :], in_=t1[:, :, :],
                                 func=mybir.ActivationFunctionType.Exp,
                                 scale=-0.5)
            # f = y * rn; rn row broadcast across partitions via K=1 ones
            # matmul (PSUM tile shares the "g" tag: lifetimes are disjoint)
            bc = gpool.tile([128, 2, S], F32, tag="g")
            ftiles = []
            for bi in range(2):
                nc.tensor.matmul(
                    bc[0:Co, bi, :], lhsT=hall[0:1, 0:Co], rhs=rn[:, bi, :],
                    start=True, stop=True,
                )
                f = fpool.tile([128, S], F16, tag="f")
                nc.vector.tensor_mul(out=f[0:Co, :], in0=ytiles[bi][0:Co, :],
                                     in1=bc[0:Co, bi, :])
                ftiles.append(f)

            fq_t, fk_t = ftiles
            # sum of positive logits: sum_s <f_q[:,s], f_k[:,s]>
            pprod = work.tile([128, S], F16, tag="pprod")
            nc.gpsimd.tensor_mul(out=pprod[0:Co, :], in0=fq_t[0:Co, :],
                                 in1=fk_t[0:Co, :])
            psum_pos = gpool.tile([1, 2, S], F32, tag="small")
            nc.tensor.matmul(psum_pos[:, 0, :], lhsT=hall[0:Co, 0:1],
                             rhs=pprod[0:Co, :], start=True, stop=True)
            nc.vector.reduce_sum(out=catb[:, 16 + l:17 + l],
                                 in_=psum_pos[:, 0, :],
                                 axis=mybir.AxisListType.X)
            # Gram tiles two at a time; one exp + one row-sum reduce per pair
            for half in range(2):
                g = gpool.tile([128, 2, S], F32, tag="g")
                for mm in range(2):
                    m = half * 2 + mm
                    nc.tensor.matmul(
                        g[:, mm, :],
                        lhsT=fq_t[0:Co, m * 128:(m + 1) * 128],
                        rhs=fk_t[0:Co, :],
                        start=True, stop=True,
                    )
                E = work.tile([128, 2, S], F32, tag="E")
                nc.scalar.activation(
                    out=E[:, :, :], in_=g[:, :, :],
                    func=mybir.ActivationFunctionType.Exp,
                    scale=1.0 / TAU,
                )
                i = l * 4 + half * 2
                nc.vector.reduce_sum(out=ZD[:, i:i + 2], in_=E[:, :, :],
                                     axis=mybir.AxisListType.X)

        if phase_split == "full":
            yt = {}
            for l in layer_order:
                yt[(l, 0)] = emit_mlp(l, 0)
                yt[(l, 1)] = emit_mlp(l, 1)
            for l in layer_order:
                emit_tail(l, [yt[(l, 0)], yt[(l, 1)]])
        elif phase_split == "pipe1":
            # software pipeline: each layer's tail is emitted after the NEXT
            # layer's MLPs so the PE always has MLP matmuls to fill tail stalls
            pend = None
            for l in layer_order:
                ytiles = [emit_mlp(l, 0), emit_mlp(l, 1)]
                if pend is not None:
                    emit_tail(*pend)
                pend = (l, ytiles)
            emit_tail(*pend)
        else:
            for l in layer_order:
                ytiles = [emit_mlp(l, 0), emit_mlp(l, 1)]
                emit_tail(l, ytiles)

        # total = sum_{p,i} ln(Z) - (1/tau) * sum_l pos_l
        L = const.tile([128, 16], F32)
        nc.scalar.activation(out=L[:, :], in_=ZD[:, :],
                             func=mybir.ActivationFunctionType.Ln)
        if fine_psum:
            tp = gpool.tile([1, S], F32, tag="small")
            tpv = tp[:, 0:16]
        else:
            tp3 = gpool.tile([1, 2, S], F32, tag="small")
            tpv = tp3[:, 0, 0:16]
        nc.tensor.matmul(tpv, lhsT=ones_col, rhs=L[:, :],
                         start=True, stop=True)
        nc.vector.tensor_copy(out=catb[:, 0:16], in_=tpv)
        wprod = const.tile([1, 20], F32)
        nc.vector.tensor_mul(out=wprod[:, :], in0=catb[:, :],
                             in1=aall[0:1, WVCOL:WVCOL + 20])
        res = const.tile([1, 1], F32)
        nc.vector.reduce_sum(out=res[:, :], in_=wprod[:, :], axis=mybir.AxisListType.X)
        nc.sync.dma_start(out=out[:, :], in_=res[:, :])
    # bass2jax's PJRT path serializes nc.m directly without finalizing;
    # Bacc's legalization passes (matmul wait splitting, register
    # allocation) only run inside finalize().
    nc.finalize()
    return nc


_NC_CACHE = {}


BEST_KW = dict(fine_psum=True, layer_order=(1, 2, 0, 3))


def _get_nc(dt_x=F16):
    key = str(dt_x)
    if key not in _NC_CACHE:
        _NC_CACHE[key] = _build_nc(dt_x, **BEST_KW)
    return _NC_CACHE[key]


def _host_blobs(inputs, np_dt=np.float16):
    """Build the shared wts/aux blobs and the per-core xq/xk blobs."""
    # gather indices per layer (host-side indexing only)
    nidx, cidx = [], []
    for l in range(4):
        sid = np.asarray(inputs[f"sid{l}"]).astype(np.int64)
        nidx.append(((sid[:, 0:1] + _DH) * 32 + (sid[:, 1:2] + _DW)).reshape(-1))
        cidx.append((sid[:, 0] + 1) * 32 + (sid[:, 1] + 1))

    wts = np.zeros((128, WTOT), dtype=np_dt)
    aux = np.zeros((128, AUXW), dtype=np.float32)
    for l in range(4):
        w1T = np.asarray(inputs[f"w1_{l}"]).astype(np.float32).T  # [Cin, Cout]
        w2T = np.asarray(inputs[f"w2_{l}"]).astype(np.float32).T  # [Cin, Cout/4]
        b1 = np.asarray(inputs[f"b1_{l}"]).astype(np.float32)
        b2 = np.asarray(inputs[f"b2_{l}"]).astype(np.float32)
        C, Co = CS[l], COUT[l]
        for kk in range(KC[l]):
            rows = min(128, C - kk * 128)
            c0 = W1C[(l, kk)]
            wts[0:rows, c0:c0 + C] = w1T[kk * 128:kk * 128 + rows, :]
            c0 = W2C[(l, kk)]
            wts[0:rows, c0:c0 + Co] = w2T[kk * 128:kk * 128 + rows, :]
        for m in range(KC[l]):
            rows = min(128, C - m * 128)
            aux[0:rows, B1C[(l, m)]] = b1[m * 128:m * 128 + rows]
        aux[0:Co, B2C[l]] = b2
    aux[:, ICOL:ICOL + 128] = np.eye(128, dtype=np.float32)
    aux[:, OCOL:OCOL + 128] = 1.0
    aux[0, WVCOL:WVCOL + 16] = 1.0
    aux[0, WVCOL + 16:WVCOL + 20] = -1.0 / TAU

    # per-core x blobs: [NCH*128, 576] = packed [neigh | center] per K chunk
    xqs = [np.zeros((NCH * 128, 576), dtype=np_dt) for _ in range(NCORES)]
    xks = [np.zeros((NCH * 128, 576), dtype=np_dt) for _ in range(NCORES)]
    for l in range(4):
        C = CS[l]
        fq = np.asarray(inputs[f"fq{l}"])[:, :, :32, :32].reshape(NCORES, C, 1024)
        fk = np.asarray(inputs[f"fk{l}"])[:, :, :32, :32].reshape(NCORES, C, 1024)
        qn = fq[:, :, nidx[l]]   # [B, C, 512]
        qc = fq[:, :, cidx[l]]   # [B, C, 64]
        kn = fk[:, :, nidx[l]]
        kc_ = fk[:, :, cidx[l]]
        for b in range(NCORES):
            for kk in range(KC[l]):
                r0 = CHUNK[(l, kk)] * 128
                rows = min(128, C - kk * 128)
                sl = slice(kk * 128, kk * 128 + rows)
                xqs[b][r0:r0 + rows, 0:512] = qn[b, sl, :]
                xqs[b][r0:r0 + rows, 512:576] = qc[b, sl, :]
                xks[b][r0:r0 + rows, 0:512] = kn[b, sl, :]
                xks[b][r0:r0 + rows, 512:576] = kc_[b, sl, :]
    return wts, aux, xqs, xks


_LAST_RESULT = {}


def kernel(**inputs):
    assert int(inputs.get("start_layer", 0)) == 0
    assert int(inputs.get("end_layer", 4)) == 4
    assert int(inputs.get("num_s", 64)) == 64

    nc = _get_nc(F16)
    wts, aux, xqs, xks = _host_blobs(inputs, np.float16)
    auxh = np.ones((128, 128), dtype=np.float16)
    in_maps = [
        {"xq": xqs[b], "xk": xks[b], "wts": wts, "aux": aux, "auxh": auxh}
        for b in range(NCORES)
    ]
    r = run_bass_kernel_spmd(nc, in_maps, core_ids=list(range(NCORES)))
    _LAST_RESULT["r"] = r
    partials = [np.float64(r.results[b]["out"][0, 0]) for b in range(NCORES)]
    loss = np.float32(sum(partials) / (NCORES * S))
    return np.asarray(loss, dtype=np.float32)


# revision 51
# speedup vs baseline: 1.0072x; 1.0072x over previous
"""CCPL contrastive loss kernel for Trainium2 (8 NeuronCores, SPMD data-parallel over batch).

Contract: kernel(**inputs) takes the FULL unsharded inputs and returns the FULL
scalar loss (float32, shape ()).

Strategy
--------
Only the top-left 32x32 corner of each feature map is ever read (sid in [0,30),
neighborhood offsets in {0,1,2}).  The host performs *indexing only* (gather of
neighbor/center columns from the corner; no arithmetic) and uploads, per core:

  xq, xk : [8*128, 576] packed K-chunks of [neigh(512) | center(64)] columns
  wts    : [128, WTOT]  packed transposed MLP weights (w1T / w2T chunks)
  aux    : [128, 268]   b1/b2 columns, identity block, ones block

Core b processes batch b end-to-end on device (fp16 matmul operands,
fp32 PSUM/softmax math):
  x = neigh - center                  (VectorE, stride-0 broadcast AP)
  h = relu(w1 @ x + b1)               (TensorE + ScalarE relu w/ bias AP)
  y = w2 @ h + b2                     (TensorE + VectorE tensor_scalar)
  f = y / ||y||_2                     (ones-matmul partition reduction;
                                       1/sqrt via ScalarE exp(-0.5*ln) --
                                       single exp/ln/relu ACT table set)
  G = f_q^T f_k                       (TensorE; |G|<=1 so exp needs no max)
  sum_s CE[s] = sum ln(sum_t exp(G/tau)) - (1/tau) sum <f_q[:,s],f_k[:,s]>
Layers processed big-first; per-layer tails rotate through fine-grained
1-bank PSUM tiles so the 4 layers' chains pipeline. Per-core partial CE
sum returned as [1,1]; host sums 8 partials / (8*512).
"""

import numpy as np
from contextlib import ExitStack

import concourse.bass as bass
import concourse.bacc as bacc
import concourse.tile as tile
from concourse import mybir
from concourse.bass_utils import run_bass_kernel_spmd

F32 = mybir.dt.float32
F16 = mybir.dt.float16

# Force Exp/Ln/Relu to resolve to the one table set that contains all three
# (natural_log_exp_and_others), so the kernel pays a single ACT_TABLE_LOAD
# instead of thrashing between exp_and_others and natural_log (~1.3us each).
# Set ids stay aligned with act_info.json because only set CONTENTS are
# filtered, never the ordering.
_COMBINED_SET = "natural_log_exp_and_others"
_orig_get_tables = bacc.get_activation_tables


def _patched_get_tables(arch):
    t = _orig_get_tables(arch)
    strip = {
        mybir.ActivationFunctionType.Exp,
        mybir.ActivationFunctionType.Ln,
        mybir.ActivationFunctionType.Relu,
    }
    return {
        name: (fns if name == _COMBINED_SET else (set(fns) - strip))
        for name, fns in t.items()
    }


bacc.get_activation_tables = _patched_get_tables

TAU = 0.07
NCORES = 8
S = 512          # 8 * num_s samples per batch-layer
NS = 64          # num_s
CS = [64, 128, 256, 512]
COUT = [16, 32, 64, 128]
KC = [1, 1, 2, 4]                 # 128-row K chunks per layer
NCH = sum(KC)                     # 8 chunks total in the x blob
_DH = np.array([0, 0, 0, 1, 1, 2, 2, 2], dtype=np.int64)
_DW = np.array([0, 1, 2, 0, 2, 0, 1, 2], dtype=np.int64)

# chunk bookkeeping -----------------------------------------------------------
CHUNK = {}
_c = 0
for _l in range(4):
    for _kk in range(KC[_l]):
        CHUNK[(_l, _kk)] = _c
        _c += 1

# weight blob column offsets, big layers first so the layer-3 block can be
# DMA'd ahead of the rest (it is needed first)
W1C, W2C = {}, {}
_c = 0
for _l in (3, 2, 1, 0):
    for _kk in range(KC[_l]):
        W1C[(_l, _kk)] = _c
        _c += CS[_l]
    for _kk in range(KC[_l]):
        W2C[(_l, _kk)] = _c
        _c += COUT[_l]
    if _l == 3:
        WSPLIT = _c          # end of the layer-3 weight block
WTOT = _c
# contiguous [start, end) column range of each layer's weight block
WBLK = {}
for _l in range(4):
    _s = W1C[(_l, 0)]
    _e = W2C[(_l, KC[_l] - 1)] + COUT[_l]
    WBLK[_l] = (_s, _e)

# aux blob layout (f32): cols 0..7 b1 chunks, 8..11 b2, 12..139 I128, 140..267 ones
B1C = {}
_c = 0
for _l in range(4):
    for _m in range(KC[_l]):
        B1C[(_l, _m)] = _c
        _c += 1
B2C = {l: 8 + l for l in range(4)}
ICOL = 12
OCOL = 140
WVCOL = 268          # row 0: [1.0]*16 | [-1/tau]*4  (final combine weights)
AUXW = 288
# auxh (fp16): cols 0..127 all-ones block; for l in 0..2 (Cout<=64, so both
# branches stack in partition bands [0:Co] and [64:64+Co]):
#   S2C[l]  : 2 cols  -- [128,2] selector, col b = ones on band b
#   BSC[l]  : 128 cols -- [2,128] selector in rows 0..1, row b = ones on band b
S2C = {l: 128 + 2 * l for l in range(3)}
BSC = {l: 134 + 128 * l for l in range(3)}
AUXH_W = 134 + 3 * 128


def _auxh():
    a = np.zeros((128, AUXH_W), dtype=np.float16)
    a[:, 0:128] = 1.0
    for l in range(3):
        Co = COUT[l]
        a[0:Co, S2C[l]] = 1.0
        a[64:64 + Co, S2C[l] + 1] = 1.0
        a[0, BSC[l]:BSC[l] + Co] = 1.0
        a[1, BSC[l] + 64:BSC[l] + 64 + Co] = 1.0
    return a


def _build_nc(dt_x=F16, mm1_bufs=2, mm2_bufs=2, dma_spread=False,
              layer_order=(3, 2, 1, 0), phase_split=False, fine_psum=False):
    nc = bacc.Bacc()
    xq = nc.dram_tensor("xq", [NCH * 128, 576], dt_x, kind="ExternalInput")
    xk = nc.dram_tensor("xk", [NCH * 128, 576], dt_x, kind="ExternalInput")
    wts = nc.dram_tensor("wts", [128, WTOT], dt_x, kind="ExternalInput")
    aux = nc.dram_tensor("aux", [128, AUXW], F32, kind="ExternalInput")
    auxh = nc.dram_tensor("auxh", [128, AUXH_W], F16, kind="ExternalInput")
    out = nc.dram_tensor("out", [1, 1], F32, kind="ExternalOutput")

    with ExitStack() as ctx:
        tc = ctx.enter_context(tile.TileContext(nc))
        const = ctx.enter_context(tc.tile_pool(name="const", bufs=1))
        work = ctx.enter_context(tc.tile_pool(name="work", bufs=2))
        hpool = ctx.enter_context(tc.tile_pool(name="hpool", bufs=3))
        ypool = ctx.enter_context(tc.tile_pool(name="ypool", bufs=6))
        fpool = ctx.enter_context(tc.tile_pool(name="fpool", bufs=6))
        # PSUM budget is 8 banks total:
        # mm1 (1 bank) * mm1_bufs + mm2 (1 bank) * mm2_bufs
        # + gpool bufs=1 * (small [1,2,512] 2 banks + g [128,2,512] 2 banks)
        ppool = ctx.enter_context(
            tc.tile_pool(name="psum", bufs=mm1_bufs, space="PSUM"))
        p2pool = ctx.enter_context(
            tc.tile_pool(name="psum2", bufs=mm2_bufs, space="PSUM"))
        gpool = ctx.enter_context(tc.tile_pool(
            name="gpsum", bufs=(2 if fine_psum else 1), space="PSUM"))

        xq_s = const.tile([128, NCH, 576], dt_x)
        xk_s = const.tile([128, NCH, 576], dt_x)
        wall = const.tile([128, WTOT], dt_x)
        aall = const.tile([128, AUXW], F32)
        hall = const.tile([128, AUXH_W], F16)
        rq = xq.rearrange("(n p) m -> p n m", p=128)
        rk = xk.rearrange("(n p) m -> p n m", p=128)
        if dma_spread:
            nc.sync.dma_start(out=xq_s[:, 4:8, :], in_=rq[:, 4:8, :])
            nc.scalar.dma_start(out=xk_s[:, 4:8, :], in_=rk[:, 4:8, :])
            nc.gpsimd.dma_start(out=wall, in_=wts[:, :])
            nc.sync.dma_start(out=xq_s[:, 0:4, :], in_=rq[:, 0:4, :])
            nc.scalar.dma_start(out=xk_s[:, 0:4, :], in_=rk[:, 0:4, :])
            nc.gpsimd.dma_start(out=aall, in_=aux[:, :])
            nc.gpsimd.dma_start(out=hall, in_=auxh[:, :])
        else:
            # land the first-processed layer's x chunk + weight block first,
            # then the rest in processing order
            fl = layer_order[0]
            c0, c1 = CHUNK[(fl, 0)], CHUNK[(fl, 0)] + KC[fl]
            w0, w1_ = WBLK[fl]
            nc.sync.dma_start(out=xq_s[:, c0:c0 + 1, :], in_=rq[:, c0:c0 + 1, :])
            nc.sync.dma_start(out=wall[:, w0:w1_], in_=wts[:, w0:w1_])
            nc.sync.dma_start(out=xk_s[:, c0:c0 + 1, :], in_=rk[:, c0:c0 + 1, :])
            if c1 > c0 + 1:
                nc.sync.dma_start(out=xq_s[:, c0 + 1:c1, :], in_=rq[:, c0 + 1:c1, :])
                nc.sync.dma_start(out=xk_s[:, c0 + 1:c1, :], in_=rk[:, c0 + 1:c1, :])
            nc.sync.dma_start(out=aall, in_=aux[:, :])
            nc.sync.dma_start(out=hall, in_=auxh[:, :])
            for l in layer_order[1:]:
                a0, a1 = CHUNK[(l, 0)], CHUNK[(l, 0)] + KC[l]
                b0, b1_ = WBLK[l]
                nc.sync.dma_start(out=wall[:, b0:b1_], in_=wts[:, b0:b1_])
                nc.sync.dma_start(out=xq_s[:, a0:a1, :], in_=rq[:, a0:a1, :])
                nc.sync.dma_start(out=xk_s[:, a0:a1, :], in_=rk[:, a0:a1, :])

        ones_col = aall[:, OCOL:OCOL + 1]
        # Z (row sums of exp(G/tau)) per G row-tile, one column per tile
        ZD = const.tile([128, 16], F32)
        # catb: cols 0..15 = per-tile sums of ln(Z); cols 16..19 = per-layer
        # sums of l_pos = sum(f_q * f_k)
        catb = const.tile([1, 20], F32)

        # x = neigh - center, four chunks per fused DVE op (center broadcast
        # over the 8 neighbors via a stride-0 trailing AP dim)
        xsub = {}
        sub_slices = []
        for li, l in enumerate(layer_order):
            a0, a1 = CHUNK[(l, 0)], CHUNK[(l, 0)] + KC[l]
            if li == 0:
                # first layer chunk-at-a-time so its first MLP matmul can
                # start as soon as the first chunk has landed
                sub_slices += [slice(c, c + 1) for c in range(a0, a1)]
            else:
                sub_slices.append(slice(a0, a1))
        for bi, xall in enumerate((xq_s, xk_s)):
            xs = const.tile([128, NCH, S], dt_x, tag=f"xsub{bi}")
            for csl in sub_slices:
                in0 = xall[:, csl, 0:512].rearrange("p n (s j) -> p n s j", j=8)
                cb = xall[:, csl, 512:576]
                in1 = bass.AP(cb.tensor, cb.offset, [*cb.ap, [0, 8]])
                nc.vector.tensor_sub(
                    out=xs[:, csl, :].rearrange("p n (s j) -> p n s j", j=8),
                    in0=in0,
                    in1=in1,
                )
            xsub[bi] = xs

        def emit_mlp(l, bi):
            C, Co, K = CS[l], COUT[l], KC[l]
            xs = xsub[bi]
            h = hpool.tile([128, K, S], dt_x, tag="h")
            for m in range(K):
                rows = min(128, C - m * 128)
                mm1 = ppool.tile([128, S], F32, tag="mm1")
                for kk in range(K):
                    c0 = W1C[(l, kk)] + m * 128
                    nc.tensor.matmul(
                        mm1[0:rows, :],
                        lhsT=wall[:, c0:c0 + rows],
                        rhs=xs[:, CHUNK[(l, kk)], :],
                        start=(kk == 0),
                        stop=(kk == K - 1),
                    )
                bc1 = B1C[(l, m)]
                nc.scalar.activation(
                    out=h[0:rows, m, :],
                    in_=mm1[0:rows, :],
                    func=mybir.ActivationFunctionType.Relu,
                    bias=aall[0:rows, bc1:bc1 + 1],
                    scale=1.0,
                )
            mm2 = p2pool.tile([128, S], F32, tag="mm2")
            for kk in range(K):
                rows = min(128, C - kk * 128)
                c0 = W2C[(l, kk)]
                nc.tensor.matmul(
                    mm2[0:Co, :],
                    lhsT=wall[0:rows, c0:c0 + Co],
                    rhs=h[0:rows, kk, :],
                    start=(kk == 0),
                    stop=(kk == K - 1),
                )
            y = ypool.tile([128, S], F32, tag="y")
            nc.vector.tensor_scalar_add(
                out=y[0:Co, :], in0=mm2[0:Co, :],
                scalar1=aall[0:Co, B2C[l]:B2C[l] + 1],
            )
            return y

        def emit_tail_fine(l, ytiles):
            C, Co, K = CS[l], COUT[l], KC[l]
            if l != 3:
                # both branches stacked in partition bands: one selector
                # matmul for both norms, one 2-row ACT chain, one broadcast
                y2 = work.tile([128, S], F16, tag="y2")
                # zero first: stale SBUF outside the two written bands
                # would otherwise poison the selector contraction (compute
                # partition starts must be 0/32/64, so no banded memset)
                nc.gpsimd.memset(y2[:, :], 0.0)
                nc.gpsimd.tensor_mul(out=y2[0:Co, :], in0=ytiles[0][0:Co, :],
                                     in1=ytiles[0][0:Co, :])
                nc.gpsimd.tensor_mul(out=y2[64:64 + Co, :],
                                     in0=ytiles[1][0:Co, :],
                                     in1=ytiles[1][0:Co, :])
                ssq = gpool.tile([2, S], F32, tag="small")
                nc.tensor.matmul(
                    ssq[:, :], lhsT=hall[:, S2C[l]:S2C[l] + 2],
                    rhs=y2[:, :], start=True, stop=True,
                )
                t1 = work.tile([2, S], F32, tag="t1")
                nc.scalar.activation(out=t1[:, :], in_=ssq[:, :],
                                     func=mybir.ActivationFunctionType.Ln)
                rn = work.tile([2, S], F16, tag="rn")
                nc.scalar.activation(out=rn[:, :], in_=t1[:, :],
                                     func=mybir.ActivationFunctionType.Exp,
                                     scale=-0.5)
                bc = gpool.tile([128, S], F32, tag="gbc")
                nc.tensor.matmul(
                    bc[:, :], lhsT=hall[0:2, BSC[l]:BSC[l] + 128],
                    rhs=rn[:, :], start=True, stop=True,
                )
                ftiles = []
                for bi in range(2):
                    f = fpool.tile([128, S], F16, tag="f")
                    nc.vector.tensor_mul(
                        out=f[0:Co, :], in0=ytiles[bi][0:Co, :],
                        in1=bc[bi * 64:bi * 64 + Co, :])
                    ftiles.append(f)
                fq_t, fk_t = ftiles
                return _tail_nce(l, Co, fq_t, fk_t)
            rns = []
            for bi in range(2):
                y2 = work.tile([128, S], F16, tag="y2")
                nc.gpsimd.tensor_mul(out=y2[0:Co, :], in0=ytiles[bi][0:Co, :],
                                     in1=ytiles[bi][0:Co, :])
                ssq = gpool.tile([1, S], F32, tag="small")
                nc.tensor.matmul(
                    ssq[:, :], lhsT=hall[0:Co, 0:1], rhs=y2[0:Co, :],
                    start=True, stop=True,
                )
                t1 = work.tile([1, S], F32, tag="t1")
                nc.scalar.activation(out=t1[:, :], in_=ssq[:, :],
                                     func=mybir.ActivationFunctionType.Ln)
                rn = work.tile([1, S], F16, tag="rn")
                nc.scalar.activation(out=rn[:, :], in_=t1[:, :],
                                     func=mybir.ActivationFunctionType.Exp,
                                     scale=-0.5)
                rns.append(rn)
            ftiles = []
            for bi in range(2):
                bc = gpool.tile([128, S], F32, tag="gbc")
                nc.tensor.matmul(
                    bc[0:Co, :], lhsT=hall[0:1, 0:Co], rhs=rns[bi][:, :],
                    start=True, stop=True,
                )
                f = fpool.tile([128, S], F16, tag="f")
                nc.vector.tensor_mul(out=f[0:Co, :], in0=ytiles[bi][0:Co, :],
                                     in1=bc[0:Co, :])
                ftiles.append(f)
            fq_t, fk_t = ftiles
            return _tail_nce(l, Co, fq_t, fk_t)

        def _tail_nce(l, Co, fq_t, fk_t):
            pprod = work.tile([128, S], F16, tag="pprod")
            nc.gpsimd.tensor_mul(out=pprod[0:Co, :], in0=fq_t[0:Co, :],
                                 in1=fk_t[0:Co, :])
            psum_pos = gpool.tile([1, S], F32, tag="small")
            nc.tensor.matmul(psum_pos[:, :], lhsT=hall[0:Co, 0:1],
                             rhs=pprod[0:Co, :], start=True, stop=True)
            nc.vector.reduce_sum(out=catb[:, 16 + l:17 + l],
                                 in_=psum_pos[:, :],
                                 axis=mybir.AxisListType.X)
            for m in range(4):
                g = gpool.tile([128, S], F32, tag="gbc")
                nc.tensor.matmul(
                    g[:, :],
                    lhsT=fq_t[0:Co, m * 128:(m + 1) * 128],
                    rhs=fk_t[0:Co, :],
                    start=True, stop=True,
                )
                E = work.tile([128, S], F32, tag="E")
                nc.scalar.activation(
                    out=E[:, :], in_=g[:, :],
                    func=mybir.ActivationFunctionType.Exp,
                    scale=1.0 / TAU,
                )
                i = l * 4 + m
                nc.vector.reduce_sum(out=ZD[:, i:i + 1], in_=E[:, :],
                                     axis=mybir.AxisListType.X)


        def emit_tail(l, ytiles):
            if fine_psum:
                return emit_tail_fine(l, ytiles)
            C, Co, K = CS[l], COUT[l], KC[l]
            # squared col norms of both branches packed in the free dim of
            # one [1, 2, 512] PSUM tile (2 banks, both MMs partition-base 0)
            ssq = gpool.tile([1, 2, S], F32, tag="small")
            for bi in range(2):
                y2 = work.tile([128, S], F16, tag="y2")
                nc.gpsimd.tensor_mul(out=y2[0:Co, :], in0=ytiles[bi][0:Co, :],
                                     in1=ytiles[bi][0:Co, :])
                nc.tensor.matmul(
                    ssq[:, bi, :], lhsT=hall[0:Co, 0:1], rhs=y2[0:Co, :],
                    start=True, stop=True,
                )
            # rn = 1/sqrt(ssq) = exp(-0.5*ln(ssq)), both branches per ACT op
            t1 = work.tile([1, 2, S], F32, tag="t1")
            nc.scalar.activation(out=t1[:, :, :], in_=ssq[:, :, :],
                                 func=mybir.ActivationFunctionType.Ln)
            rn = work.tile([1, 2, S], F16, tag="rn")
            nc.scalar.activation(out=rn[:, :, :], in_=t1[:, :, :],
                                 func=mybir.ActivationFunctionType.Exp,
                                 scale=-0.5)
            # f = y * rn; rn row broadcast across partitions via K=1 ones
            # matmul (PSUM tile shares the "g" tag: lifetimes are disjoint)
            bc = gpool.tile([128, 2, S], F32, tag="g")
            ftiles = []
            for bi in range(2):
                nc.tensor.matmul(
                    bc[0:Co, bi, :], lhsT=hall[0:1, 0:Co], rhs=rn[:, bi, :],
                    start=True, stop=True,
                )
                f = fpool.tile([128, S], F16, tag="f")
                nc.vector.tensor_mul(out=f[0:Co, :], in0=ytiles[bi][0:Co, :],
                                     in1=bc[0:Co, bi, :])
                ftiles.append(f)

            fq_t, fk_t = ftiles
            # sum of positive logits: sum_s <f_q[:,s], f_k[:,s]>
            pprod = work.tile([128, S], F16, tag="pprod")
            nc.gpsimd.tensor_mul(out=pprod[0:Co, :], in0=fq_t[0:Co, :],
                                 in1=fk_t[0:Co, :])
            psum_pos = gpool.tile([1, 2, S], F32, tag="small")
            nc.tensor.matmul(psum_pos[:, 0, :], lhsT=hall[0:Co, 0:1],
                             rhs=pprod[0:Co, :], start=True, stop=True)
            nc.vector.reduce_sum(out=catb[:, 16 + l:17 + l],
                                 in_=psum_pos[:, 0, :],
                                 axis=mybir.AxisListType.X)
            # Gram tiles two at a time; one exp + one row-sum reduce per pair
            for half in range(2):
                g = gpool.tile([128, 2, S], F32, tag="g")
                for mm in range(2):
                    m = half * 2 + mm
                    nc.tensor.matmul(
                        g[:, mm, :],
                        lhsT=fq_t[0:Co, m * 128:(m + 1) * 128],
                        rhs=fk_t[0:Co, :],
                        start=True, stop=True,
                    )
                E = work.tile([128, 2, S], F32, tag="E")
                nc.scalar.activation(
                    out=E[:, :, :], in_=g[:, :, :],
                    func=mybir.ActivationFunctionType.Exp,
                    scale=1.0 / TAU,
                )
                i = l * 4 + half * 2
                nc.vector.reduce_sum(out=ZD[:, i:i + 2], in_=E[:, :, :],
                                     axis=mybir.AxisListType.X)

        if phase_split == "full":
            yt = {}
            for l in layer_order:
                yt[(l, 0)] = emit_mlp(l, 0)
                yt[(l, 1)] = emit_mlp(l, 1)
            for l in layer_order:
                emit_tail(l, [yt[(l, 0)], yt[(l, 1)]])
        elif phase_split == "pipe1":
            # software pipeline: each layer's tail is emitted after the NEXT
            # layer's MLPs so the PE always has MLP matmuls to fill tail stalls
            pend = None
            for l in layer_order:
                ytiles = [emit_mlp(l, 0), emit_mlp(l, 1)]
                if pend is not None:
                    emit_tail(*pend)
                pend = (l, ytiles)
            emit_tail(*pend)
        else:
            for l in layer_order:
                ytiles = [emit_mlp(l, 0), emit_mlp(l, 1)]
                emit_tail(l, ytiles)

        # total = sum_{p,i} ln(Z) - (1/tau) * sum_l pos_l
        L = const.tile([128, 16], F32)
        nc.scalar.activation(out=L[:, :], in_=ZD[:, :],
                             func=mybir.ActivationFunctionType.Ln)
        if fine_psum:
            tp = gpool.tile([1, S], F32, tag="small")
            tpv = tp[:, 0:16]
        else:
            tp3 = gpool.tile([1, 2, S], F32, tag="small")
            tpv = tp3[:, 0, 0:16]
        nc.tensor.matmul(tpv, lhsT=ones_col, rhs=L[:, :],
                         start=True, stop=True)
        nc.vector.tensor_copy(out=catb[:, 0:16], in_=tpv)
        wprod = const.tile([1, 20], F32)
        nc.vector.tensor_mul(out=wprod[:, :], in0=catb[:, :],
                             in1=aall[0:1, WVCOL:WVCOL + 20])
        res = const.tile([1, 1], F32)
        nc.vector.reduce_sum(out=res[:, :], in_=wprod[:, :], axis=mybir.AxisListType.X)
        nc.sync.dma_start(out=out[:, :], in_=res[:, :])
    # bass2jax's PJRT path serializes nc.m directly without finalizing;
    # Bacc's legalization passes (matmul wait splitting, register
    # allocation) only run inside finalize().
    nc.finalize()
    return nc


_NC_CACHE = {}


BEST_KW = dict(fine_psum=True, layer_order=(1, 2, 0, 3))


def _get_nc(dt_x=F16):
    key = str(dt_x)
    if key not in _NC_CACHE:
        _NC_CACHE[key] = _build_nc(dt_x, **BEST_KW)
    return _NC_CACHE[key]


def _host_blobs(inputs, np_dt=np.float16):
    """Build the shared wts/aux blobs and the per-core xq/xk blobs."""
    # gather indices per layer (host-side indexing only)
    nidx, cidx = [], []
    for l in range(4):
        sid = np.asarray(inputs[f"sid{l}"]).astype(np.int64)
        nidx.append(((sid[:, 0:1] + _DH) * 32 + (sid[:, 1:2] + _DW)).reshape(-1))
        cidx.append((sid[:, 0] + 1) * 32 + (sid[:, 1] + 1))

    wts = np.zeros((128, WTOT), dtype=np_dt)
    aux = np.zeros((128, AUXW), dtype=np.float32)
    for l in range(4):
        w1T = np.asarray(inputs[f"w1_{l}"]).astype(np.float32).T  # [Cin, Cout]
        w2T = np.asarray(inputs[f"w2_{l}"]).astype(np.float32).T  # [Cin, Cout/4]
        b1 = np.asarray(inputs[f"b1_{l}"]).astype(np.float32)
        b2 = np.asarray(inputs[f"b2_{l}"]).astype(np.float32)
        C, Co = CS[l], COUT[l]
        for kk in range(KC[l]):
            rows = min(128, C - kk * 128)
            c0 = W1C[(l, kk)]
            wts[0:rows, c0:c0 + C] = w1T[kk * 128:kk * 128 + rows, :]
            c0 = W2C[(l, kk)]
            wts[0:rows, c0:c0 + Co] = w2T[kk * 128:kk * 128 + rows, :]
        for m in range(KC[l]):
            rows = min(128, C - m * 128)
            aux[0:rows, B1C[(l, m)]] = b1[m * 128:m * 128 + rows]
        aux[0:Co, B2C[l]] = b2
    aux[:, ICOL:ICOL + 128] = np.eye(128, dtype=np.float32)
    aux[:, OCOL:OCOL + 128] = 1.0
    aux[0, WVCOL:WVCOL + 16] = 1.0
    aux[0, WVCOL + 16:WVCOL + 20] = -1.0 / TAU

    # per-core x blobs: [NCH*128, 576] = packed [neigh | center] per K chunk
    xqs = [np.zeros((NCH * 128, 576), dtype=np_dt) for _ in range(NCORES)]
    xks = [np.zeros((NCH * 128, 576), dtype=np_dt) for _ in range(NCORES)]
    for l in range(4):
        C = CS[l]
        fq = np.asarray(inputs[f"fq{l}"])[:, :, :32, :32].reshape(NCORES, C, 1024)
        fk = np.asarray(inputs[f"fk{l}"])[:, :, :32, :32].reshape(NCORES, C, 1024)
        qn = fq[:, :, nidx[l]]   # [B, C, 512]
        qc = fq[:, :, cidx[l]]   # [B, C, 64]
        kn = fk[:, :, nidx[l]]
        kc_ = fk[:, :, cidx[l]]
        for b in range(NCORES):
            for kk in range(KC[l]):
                r0 = CHUNK[(l, kk)] * 128
                rows = min(128, C - kk * 128)
                sl = slice(kk * 128, kk * 128 + rows)
                xqs[b][r0:r0 + rows, 0:512] = qn[b, sl, :]
                xqs[b][r0:r0 + rows, 512:576] = qc[b, sl, :]
                xks[b][r0:r0 + rows, 0:512] = kn[b, sl, :]
                xks[b][r0:r0 + rows, 512:576] = kc_[b, sl, :]
    return wts, aux, xqs, xks


_LAST_RESULT = {}


def kernel(**inputs):
    assert int(inputs.get("start_layer", 0)) == 0
    assert int(inputs.get("end_layer", 4)) == 4
    assert int(inputs.get("num_s", 64)) == 64

    nc = _get_nc(F16)
    wts, aux, xqs, xks = _host_blobs(inputs, np.float16)
    auxh = _auxh()
    in_maps = [
        {"xq": xqs[b], "xk": xks[b], "wts": wts, "aux": aux, "auxh": auxh}
        for b in range(NCORES)
    ]
    r = run_bass_kernel_spmd(nc, in_maps, core_ids=list(range(NCORES)))
    _LAST_RESULT["r"] = r
    partials = [np.float64(r.results[b]["out"][0, 0]) for b in range(NCORES)]
    loss = np.float32(sum(partials) / (NCORES * S))
    return np.asarray(loss, dtype=np.float32)
